# revision 9
# baseline (speedup 1.0000x reference)
"""Trainium2 Bass kernel for nn_CrossAttentionLayer (ragged cross-attention + MLP).

v2 design (bf16, ragged-trimmed, col-tiled attention):
- 64 ragged segments sorted by estimated cost and dealt one-per-octile to the
  8 cores; one SPMD program is compiled with per-slot shapes = max over the 8
  cores in that slot (so every core runs the same program on its own data).
- All activations/weights bf16 (fp32 PSUM accumulate); inputs/outputs are
  channel-major packed per slot with slot-max column widths.
- Attention runs in scoresT orientation [src, dst]: per j-block of 128 src
  tokens, scores for the 4 heads are row-tiled (tile_position=(32h,0), K=32)
  into 2-bank PSUM tiles (head pairs, double-buffered so ACT-exp and PE
  pipeline); exp applies the src-padding mask as a per-partition bias.
- V is produced directly in natural [src_token, chan] layout (stationary =
  srcT block), so msg = V^T E and den = 1^T E are 4-way col-tiled matmuls
  (tile_position=(0,32h)) accumulating over j.
- bk drops entirely (softmax shift-invariant over dst); bv is folded into the
  MLP bias; merge conv (Wm, bm) folded into W1/b1; BN folded into W1/W2.
- Softmax normalize: reciprocal_approx_fast + one multiply; MLP residual is a
  single fused scalar_tensor_tensor (z + b2 + dst).
"""
import math
import sys
from contextlib import ExitStack

import numpy as np
import ml_dtypes

try:
    import concourse.bass as bass
except ImportError:
    sys.path.insert(0, "/opt/trn_rl_repo")
    import concourse.bass as bass

import concourse.tile as tile
from concourse import bacc, mybir
from concourse.bass_utils import run_bass_kernel_spmd

F32 = mybir.dt.float32
BF16 = mybir.dt.bfloat16
BF = ml_dtypes.bfloat16

B = 64
LMAX = 512
H = 256          # h_dim
C = 128          # h_div
HEADS = 4
DH = 32
NCORES = 8
SEGS = 8         # segments (slots) per core
NPB = 5          # per-partition bias columns: bq, b1a, b1b, b2a, b2b
MASK_NEG = -20000.0


def _plan(ns, nd):
    """Sort segments by cost, deal one per octile to each core.

    Returns (seg_of[core][slot] -> global segment id, Ldm[slot], njm[slot],
    doffs[slot], soffs[slot], T_d, T_s)."""
    nj = (ns + 127) // 128
    cost = nj * (4 * nd + 420)          # ACT-exp model drives the sort
    order = np.argsort(-cost, kind="stable")
    seg_of = [[0] * SEGS for _ in range(NCORES)]
    Ldm = np.zeros(SEGS, np.int64)
    njm = np.zeros(SEGS, np.int64)
    for k in range(SEGS):
        grp = order[k * NCORES:(k + 1) * NCORES]
        for c in range(NCORES):
            seg_of[c][k] = int(grp[c])
        Ldm[k] = -(-int(nd[grp].max()) // 8) * 8   # round to 8 elems
        njm[k] = int(nj[grp].max())
    doffs = np.concatenate([[0], np.cumsum(Ldm)[:-1]]).astype(np.int64)
    soffs = np.concatenate([[0], np.cumsum(njm * 128)[:-1]]).astype(np.int64)
    return seg_of, Ldm, njm, doffs, soffs, int(Ldm.sum()), int((njm * 128).sum())


def host_prep(inputs):
    src_h = np.asarray(inputs['src_h'], np.float32)
    dst_h = np.asarray(inputs['dst_h'], np.float32)
    ns = np.asarray(inputs['src_num_verts']).astype(np.int64)
    nd = np.asarray(inputs['dst_num_verts']).astype(np.int64)
    assert ns.max() <= LMAX and nd.max() <= LMAX
    soff = np.concatenate([[0], np.cumsum(ns)[:-1]])
    doff = np.concatenate([[0], np.cumsum(nd)[:-1]])

    seg_of, Ldm, njm, doffs, soffs, T_d, T_s = _plan(ns, nd)

    perm = np.empty(C, np.int64)
    for chat in range(C):
        h, d = divmod(chat, DH)
        perm[chat] = d * HEADS + h
    s = 1.0 / math.sqrt(DH)

    f32 = lambda k: np.asarray(inputs[k], np.float32)
    Wq, bq = f32('Wq'), f32('bq')
    Wk = f32('Wk')
    Wv, bv = f32('Wv'), f32('bv')
    Wm, bm = f32('Wm'), f32('bm')
    W1, b1 = f32('W1'), f32('b1')
    W2, b2 = f32('W2'), f32('b2')
    g1, be1, rm1, rv1 = f32('g1'), f32('be1'), f32('rm1'), f32('rv1')
    g2, be2, rm2, rv2 = f32('g2'), f32('be2'), f32('rm2'), f32('rv2')

    WqT = np.ascontiguousarray((Wq[perm] * s).T).astype(BF)   # [256,128]
    bq_s = bq[perm] * s
    WkT = np.ascontiguousarray(Wk[perm].T).astype(BF)
    WvT = np.ascontiguousarray(Wv[perm].T).astype(BF)         # rhs for v_nat
    Wm_p = Wm[:, perm]
    a1 = g1 / np.sqrt(rv1 + 1e-5)
    W1_f = W1 * a1[:, None]
    b1_f = b1 * a1 + be1 - rm1 * a1
    a2 = g2 / np.sqrt(rv2 + 1e-5)
    W2_f = W2 * a2[:, None]
    b2_f = b2 * a2 + be2 - rm2 * a2
    W1m_p = W1_f[:, H:] @ Wm_p
    b1_p = b1_f + W1_f[:, H:] @ (bm + Wm @ bv)
    W1T = np.ascontiguousarray(
        np.concatenate([W1_f[:, :H], W1m_p], axis=1).T).astype(BF)  # [384,256]
    W2T = np.ascontiguousarray(W2_f.T).astype(BF)                   # [256,256]

    pb = np.zeros((128, NPB), np.float32)
    pb[:, 0] = bq_s
    pb[:, 1] = b1_p[:128]
    pb[:, 2] = b1_p[128:]
    pb[:, 3] = b2_f[:128]
    pb[:, 4] = b2_f[128:]

    cores = []
    for c in range(NCORES):
        dstP = np.zeros((128, 2, T_d), BF)
        srcP = np.zeros((128, 2, T_s), BF)
        maskb = np.full((128, SEGS * 4), MASK_NEG, np.float32)
        for k in range(SEGS):
            g = seg_of[c][k]
            ndg, nsg = int(nd[g]), int(ns[g])
            db = dst_h[doff[g]:doff[g] + ndg].T.astype(BF)   # [256, ndg]
            sb = src_h[soff[g]:soff[g] + nsg].T.astype(BF)
            for a in range(2):
                dstP[:, a, doffs[k]:doffs[k] + ndg] = db[a * 128:(a + 1) * 128]
                srcP[:, a, soffs[k]:soffs[k] + nsg] = sb[a * 128:(a + 1) * 128]
            for j in range(int(njm[k])):
                valid = max(0, min(128, nsg - j * 128))
                maskb[:valid, k * 4 + j] = 0.0
        cores.append(dict(dstP=dstP, srcP=srcP, maskb=maskb))

    ones32 = np.ones((128, 32), BF)
    shared = dict(WqT=WqT, WkT=WkT, WvT=WvT, W1T=W1T, W2T=W2T, pb=pb,
                  ones32=ones32)
    meta = dict(nd=nd, doff=doff, seg_of=seg_of, Ldm=Ldm, njm=njm,
                doffs=doffs, soffs=soffs, T_d=T_d, T_s=T_s)
    return cores, shared, meta


def declare_tensors(nc, meta):
    T_d, T_s = meta['T_d'], meta['T_s']
    aps = {}
    aps['dstP'] = nc.dram_tensor("dstP", [128, 2, T_d], BF16, kind="ExternalInput").ap()
    aps['srcP'] = nc.dram_tensor("srcP", [128, 2, T_s], BF16, kind="ExternalInput").ap()
    aps['maskb'] = nc.dram_tensor("maskb", [128, SEGS * 4], F32, kind="ExternalInput").ap()
    aps['WqT'] = nc.dram_tensor("WqT", [H, C], BF16, kind="ExternalInput").ap()
    aps['WkT'] = nc.dram_tensor("WkT", [H, C], BF16, kind="ExternalInput").ap()
    aps['WvT'] = nc.dram_tensor("WvT", [H, C], BF16, kind="ExternalInput").ap()
    aps['W1T'] = nc.dram_tensor("W1T", [H + C, H], BF16, kind="ExternalInput").ap()
    aps['W2T'] = nc.dram_tensor("W2T", [H, H], BF16, kind="ExternalInput").ap()
    aps['pb'] = nc.dram_tensor("pb", [128, NPB], F32, kind="ExternalInput").ap()
    aps['ones32'] = nc.dram_tensor("ones32", [128, 32], BF16, kind="ExternalInput").ap()
    aps['outP'] = nc.dram_tensor("outP", [128, 2, T_d], BF16, kind="ExternalOutput").ap()
    return aps


def build_body(ctx: ExitStack, tc: tile.TileContext, aps, meta):
    nc = tc.nc
    Ldm = [int(x) for x in meta['Ldm']]
    njm = [int(x) for x in meta['njm']]
    doffs = [int(x) for x in meta['doffs']]
    soffs = [int(x) for x in meta['soffs']]
    T_d, T_s = meta['T_d'], meta['T_s']

    wp = ctx.enter_context(tc.tile_pool(name="wp", bufs=1))
    inp = ctx.enter_context(tc.tile_pool(name="inp", bufs=1))
    qk = ctx.enter_context(tc.tile_pool(name="qk", bufs=2))
    ee = ctx.enter_context(tc.tile_pool(name="ee", bufs=3))
    nrm = ctx.enter_context(tc.tile_pool(name="nrm", bufs=2))
    ml = ctx.enter_context(tc.tile_pool(name="ml", bufs=2))
    # PSUM: sc (scores, 2 banks x2) + att (msg+den, 2 banks) + pm (proj/mlp, 2)
    scp = ctx.enter_context(tc.tile_pool(name="scp", bufs=2, space="PSUM"))
    attp = ctx.enter_context(tc.tile_pool(name="attp", bufs=1, space="PSUM"))
    pmp = ctx.enter_context(tc.tile_pool(name="pmp", bufs=2, space="PSUM"))

    # --- weights ---
    wq = wp.tile([128, 2, C], BF16, tag="wq")
    wk = wp.tile([128, 2, C], BF16, tag="wk")
    wv = wp.tile([128, 2, C], BF16, tag="wv")
    w1 = wp.tile([128, 3, H], BF16, tag="w1")
    w2 = wp.tile([128, 2, H], BF16, tag="w2")
    pb = wp.tile([128, NPB], F32, tag="pb")
    maskb = wp.tile([128, SEGS * 4], F32, tag="maskb")
    ones32 = wp.tile([128, 32], BF16, tag="ones32")
    # --- inputs + weights: slot-0 data and its weights first so proj(0)
    # starts as early as possible ---
    dst_sb = inp.tile([128, 2, T_d], BF16, tag="dst")
    src_sb = inp.tile([128, 2, T_s], BF16, tag="src")
    d0 = doffs[1] if SEGS > 1 else T_d
    s0 = soffs[1] if SEGS > 1 else T_s
    dh = doffs[4] if SEGS > 4 else T_d
    sh = soffs[4] if SEGS > 4 else T_s
    nc.sync.dma_start(out=dst_sb[:, :, 0:d0], in_=aps['dstP'][:, :, 0:d0])
    nc.sync.dma_start(out=wq[:], in_=aps['WqT'].rearrange("(a p) c -> p a c", a=2))
    nc.sync.dma_start(out=src_sb[:, :, 0:s0], in_=aps['srcP'][:, :, 0:s0])
    nc.sync.dma_start(out=wk[:], in_=aps['WkT'].rearrange("(a p) c -> p a c", a=2))
    nc.sync.dma_start(out=wv[:], in_=aps['WvT'].rearrange("(a p) c -> p a c", a=2))
    nc.sync.dma_start(out=pb[:], in_=aps['pb'][:])
    nc.sync.dma_start(out=maskb[:], in_=aps['maskb'][:])
    nc.sync.dma_start(out=dst_sb[:, :, d0:dh], in_=aps['dstP'][:, :, d0:dh])
    nc.sync.dma_start(out=src_sb[:, :, s0:sh], in_=aps['srcP'][:, :, s0:sh])
    nc.sync.dma_start(out=ones32[:], in_=aps['ones32'][:])
    nc.sync.dma_start(out=w1[:], in_=aps['W1T'].rearrange("(a p) c -> p a c", a=3))
    nc.sync.dma_start(out=w2[:], in_=aps['W2T'].rearrange("(a p) c -> p a c", a=2))
    if dh < T_d:
        nc.sync.dma_start(out=dst_sb[:, :, dh:T_d], in_=aps['dstP'][:, :, dh:T_d])
    if sh < T_s:
        nc.sync.dma_start(out=src_sb[:, :, sh:T_s], in_=aps['srcP'][:, :, sh:T_s])

    state = {}

    def proj(s):
        Ld, nj, do_, so_ = Ldm[s], njm[s], doffs[s], soffs[s]
        ls = nj * 128
        ps_q = pmp.tile([128, 512], F32, tag="pm", name=f"psq{s}")
        for a in range(2):
            nc.tensor.matmul(ps_q[:, :Ld], wq[:, a, :], dst_sb[:, a, do_:do_ + Ld],
                             start=(a == 0), stop=(a == 1))
        q_t = qk.tile([128, 512], BF16, tag="q", name=f"q{s}")
        nc.vector.tensor_scalar_add(q_t[:, :Ld], ps_q[:, :Ld], pb[:, 0:1])

        ps_k = pmp.tile([128, 512], F32, tag="pm", name=f"psk{s}")
        for a in range(2):
            nc.tensor.matmul(ps_k[:, :ls], wk[:, a, :], src_sb[:, a, so_:so_ + ls],
                             start=(a == 0), stop=(a == 1))
        k_t = qk.tile([128, 512], BF16, tag="k", name=f"k{s}")
        nc.vector.tensor_copy(k_t[:, :ls], ps_k[:, :ls])

        v_t = qk.tile([128, 4, C], BF16, tag="v", name=f"v{s}")
        ps_v = pmp.tile([128, 512], F32, tag="pm", name=f"psv{s}")
        for j in range(nj):
            for a in range(2):
                nc.tensor.matmul(ps_v[:, j * C:(j + 1) * C],
                                 src_sb[:, a, so_ + j * 128: so_ + (j + 1) * 128],
                                 wv[:, a, :], start=(a == 0), stop=(a == 1))
        nc.vector.tensor_copy(v_t[:, :nj, :], ps_v[:, :nj * C])
        state[s] = (q_t, k_t, v_t)

    def scores(s, j, hp):
        Ld = Ldm[s]
        q_t, k_t, _ = state[s]
        sc_t = scp.tile([128, 2, 512], F32, tag="sc", name=f"sc{s}_{j}_{hp}")
        for hh in range(2):
            h = 2 * hp + hh
            nc.tensor.matmul(
                sc_t[:, hh, :Ld],
                k_t[32 * h:32 * h + 32, j * 128:(j + 1) * 128],
                q_t[32 * h:32 * h + 32, :Ld],
                start=True, stop=True, tile_position=(32 * h, 0))
        return sc_t

    def attn(s):
        Ld, nj = Ldm[s], njm[s]
        ps_msg = attp.tile([128, 512], F32, tag="msg", name=f"psmsg{s}")
        ps_den = attp.tile([128, 512], F32, tag="den", name=f"psden{s}")
        sc_cur = [scores(s, 0, 0), scores(s, 0, 1)]
        for j in range(nj):
            e_t = ee.tile([128, 4, 512], BF16, tag="E", name=f"E{s}_{j}")
            for hp in range(2):
                nc.scalar.activation(e_t[:, 2 * hp:2 * hp + 2, :Ld],
                                     sc_cur[hp][:, :, :Ld],
                                     mybir.ActivationFunctionType.Exp,
                                     bias=maskb[:, s * 4 + j: s * 4 + j + 1])
                if j + 1 < nj:
                    sc_cur[hp] = scores(s, j + 1, hp)
            first, last = (j == 0), (j == nj - 1)
            for h in range(HEADS):
                nc.tensor.matmul(
                    ps_den[32 * h:32 * h + 32, :Ld],
                    ones32[:, :],
                    e_t[:, h, :Ld],
                    start=first, stop=last, tile_position=(0, 32 * h),
                    skip_group_check=True)
            for h in range(HEADS):
                nc.tensor.matmul(
                    ps_msg[32 * h:32 * h + 32, :Ld],
                    state[s][2][:, j, 32 * h:32 * h + 32],
                    e_t[:, h, :Ld],
                    start=first, stop=last, tile_position=(0, 32 * h),
                    skip_group_check=True)
        state.pop(s)
        r_t = nrm.tile([128, 512], F32, tag="r", name=f"r{s}")
        nc.vector.reciprocal_approx_fast(r_t[:, :Ld], ps_den[:, :Ld])
        msgn = nrm.tile([128, 512], BF16, tag="msgn", name=f"msgn{s}")
        nc.vector.tensor_tensor(msgn[:, :Ld], ps_msg[:, :Ld], r_t[:, :Ld],
                                mybir.AluOpType.mult)
        state[(s, 'msgn')] = msgn

    def mlp(s):
        Ld, do_ = Ldm[s], doffs[s]
        msgn = state.pop((s, 'msgn'))
        y1 = ml.tile([128, 2, 512], BF16, tag="y1", name=f"y1_{s}")
        for o in range(2):
            ps_y = pmp.tile([128, 512], F32, tag="pm", name=f"psy{s}_{o}")
            rhs = [dst_sb[:, 0, do_:do_ + Ld], dst_sb[:, 1, do_:do_ + Ld],
                   msgn[:, :Ld]]
            for kk in range(3):
                nc.tensor.matmul(ps_y[:, :Ld], w1[:, kk, o * 128:(o + 1) * 128],
                                 rhs[kk], start=(kk == 0), stop=(kk == 2))
            nc.vector.tensor_scalar(y1[:, o, :Ld], ps_y[:, :Ld],
                                    pb[:, 1 + o:2 + o], 0.0,
                                    op0=mybir.AluOpType.add,
                                    op1=mybir.AluOpType.max)
        out_sb = ml.tile([128, 2, 512], BF16, tag="out", name=f"out{s}")
        for o in range(2):
            ps_z = pmp.tile([128, 512], F32, tag="pm", name=f"psz{s}_{o}")
            for kk in range(2):
                nc.tensor.matmul(ps_z[:, :Ld], w2[:, kk, o * 128:(o + 1) * 128],
                                 y1[:, kk, :Ld], start=(kk == 0), stop=(kk == 1))
            nc.vector.scalar_tensor_tensor(
                out_sb[:, o, :Ld], ps_z[:, :Ld], pb[:, 3 + o:4 + o],
                dst_sb[:, o, do_:do_ + Ld],
                op0=mybir.AluOpType.add, op1=mybir.AluOpType.add)
        nc.sync.dma_start(out=aps['outP'][:, :, do_:do_ + Ld],
                          in_=out_sb[:, :, :Ld])

    proj(0)
    for s in range(SEGS):
        if s > 0:
            mlp(s - 1)
        if s + 1 < SEGS:
            proj(s + 1)
        attn(s)
    mlp(SEGS - 1)


def build_nc(meta, reps=1):
    nc = bacc.Bacc("TRN2", target_bir_lowering=False, debug=False,
                   enable_asserts=False, num_devices=NCORES)
    aps = declare_tensors(nc, meta)
    with tile.TileContext(nc) as tc:
        for rep in range(reps):
            with ExitStack() as ctx:
                build_body(ctx, tc, aps, meta)
    nc.compile()
    return nc


def in_map(core, shared):
    m = dict(dstP=core['dstP'], srcP=core['srcP'], maskb=core['maskb'])
    m.update({k: shared[k] for k in ('WqT', 'WkT', 'WvT', 'W1T', 'W2T', 'pb',
                                     'ones32')})
    return m


def assemble(outPs, meta):
    nd = meta['nd']
    doff = meta['doff']
    out = np.empty((int(nd.sum()), H), np.float32)
    for c in range(NCORES):
        arr = np.asarray(outPs[c], dtype=np.float32)   # [128, 2, T_d]
        for k in range(SEGS):
            g = meta['seg_of'][c][k]
            o = meta['doffs'][k]
            blk = arr[:, :, o:o + int(nd[g])]          # [128, 2, nd]
            out[doff[g]:doff[g] + nd[g]] = blk.transpose(2, 1, 0).reshape(int(nd[g]), H)
    return out


def kernel(**inputs):
    cores, shared, meta = host_prep(inputs)
    nc = build_nc(meta)
    in_maps = [in_map(cores[c], shared) for c in range(NCORES)]
    res = run_bass_kernel_spmd(nc, in_maps, core_ids=list(range(NCORES)))
    outPs = [res.results[c]["outP"] for c in range(NCORES)]
    return assemble(outPs, meta)
